# revision 48
# baseline (speedup 1.0000x reference)
"""Trainium2 Bass kernel for NT-Xent style contrastive loss (v5).

Math (B=4096, D=128, T=0.25), z = row-normalized emb:
  S = z_i @ z_j^T   [B, B]
  loss = (1/2B) * sum_r [ -2*S[r,r]/T + ln(sum_c exp(S[r,c]/T))
                                      + ln(sum_c exp(S[c,r]/T)) ]

Sharding: core (rb, ch), rb = core//2, ch = core%2: rows rb*1024 of emb_i,
cols ch*2048 of emb_j.  Inputs are cast to bf16 ON THE HOST (input
quantization, ~0.4% per element, far inside the 2e-2 tolerance): halves
the DMA wire time, feeds the matmuls directly, and makes every stats op
2x-mode eligible.  All tensors use the "(p t) d" 8-row tiling (partition
g//8, tile g%8, 2KB contiguous descriptors); outputs are unpermuted on
the host.  emb_j's block is loaded as two 1024-row halves with the same
tiling as emb_i's block, so the diagonal dot products are computed
against cj directly (each row block equals one cj half on cores
0/2/5/7) and the column norms n2c double as the diagonal |z_j| norms.

Orientation: ps chunk k = [128 r (row-tile k), 2048 c free], 8 chunks.
  stationary = aibT tile k (RAW bf16 emb_i, transposed straight off the
               load; row stats are off the matmul critical path)
  moving     = zcjT halves (cols pre-scaled by 4/|c|, absorbing 1/T)
The EXP applies the row norm via its per-partition scale AP
(scale = invi[:,k]) and its accumulator emits the ROW SUMS directly
(accum_out -> rs[:, k]).  Column-sum partials: bf16 TT esum chain
(DVE 2x, chunks 1-6) + accumulating mask matmuls over {esum, eb_7},
stacked on PSUM partitions 32q so one free-512 ACT copy extracts them.

Engine split: DVE sq_a/red_a/sq_i/red_b/scale_a/scale_b/red_i then the
esum chain (diag reduces dep-forced into loop slack); GP sq_b + diag
mults; ACT only ln/exp stat scalars + the 8 big EXPs + cs copy.
"""

import numpy as np

B = 4096
D = 128
P = 128
NCORES = 8
RB = 1024                  # rows per core
CB = 2048                  # cols per core
RT = RB // P               # 8 row tiles
TEMP = 0.25
LN4 = float(np.log(4.0))

_cache = {}


def _build_bass():
    import concourse.bass as bass
    import concourse.mybir as mybir
    import concourse.tile as tile
    from concourse.bass import broadcast_tensor_aps
    from concourse.tile_rust import add_dep_helper

    f32 = mybir.dt.float32
    bf16 = mybir.dt.bfloat16
    AF = mybir.ActivationFunctionType
    ALU = mybir.AluOpType
    AX = mybir.AxisListType

    nc = bass.Bass("TRN2")
    ai_d = nc.dram_tensor("emb_i_blk", [RB, D], bf16, kind="ExternalInput")
    ca_d = nc.dram_tensor("emb_j_ca", [RB, D], bf16, kind="ExternalInput")
    cb_d = nc.dram_tensor("emb_j_cb", [RB, D], bf16, kind="ExternalInput")
    out_rs = nc.dram_tensor("rowsum", [P, RT], f32, kind="ExternalOutput")
    out_cs = nc.dram_tensor("colsum", [P, 512], bf16, kind="ExternalOutput")
    out_dg = nc.dram_tensor("diag", [P, 5 * RT], f32, kind="ExternalOutput")

    ai_t = ai_d.rearrange("(p t) d -> p t d", p=P)   # row g = p*8 + t
    ca_t = ca_d.rearrange("(p t) d -> p t d", p=P)   # col g = p*8 + t
    cb_t = cb_d.rearrange("(p t) d -> p t d", p=P)   # col g = 1024 + p*8 + t

    with tile.TileContext(nc) as tc:
        with (
            tc.tile_pool(name="persist", bufs=1) as persist,
            tc.tile_pool(name="scratch", bufs=4) as scratch,
            tc.tile_pool(name="ebuf", bufs=2) as ebuf,
            tc.tile_pool(name="psmain", bufs=2, space="PSUM") as psmain,
        ):
            aib = persist.tile([P, RT, D], bf16, tag="aib")
            cja = persist.tile([P, RT, D], bf16, tag="cja")
            cjb = persist.tile([P, RT, D], bf16, tag="cjb")
            aibT = persist.tile([P, RT, D], bf16, tag="aibT")
            sqi = persist.tile([P, RT, D], bf16, tag="sqi")
            sqa = persist.tile([P, RT, D], bf16, tag="sqa")
            sqb = persist.tile([P, RT, D], bf16, tag="sqb")
            zca = persist.tile([P, RT, D], bf16, tag="zca")
            zcb = persist.tile([P, RT, D], bf16, tag="zcb")
            zcaT = persist.tile([P, RT, D], bf16, tag="zcaT")
            zcbT = persist.tile([P, RT, D], bf16, tag="zcbT")
            n2a = persist.tile([P, RT], f32, tag="n2a")
            n2b = persist.tile([P, RT], f32, tag="n2b")
            n2i = persist.tile([P, RT], f32, tag="n2i")
            inv4a = persist.tile([P, RT, 1], bf16, tag="inv4a")
            inv4b = persist.tile([P, RT, 1], bf16, tag="inv4b")
            dg = persist.tile([P, 5 * RT], f32, tag="dg")
            rs_sb = persist.tile([P, RT], f32, tag="rs_sb")
            cs_sb = persist.tile([P, 512], bf16, tag="cs_sb")
            esum = [
                persist.tile([P, CB], bf16, name="esum0", tag="esum0"),
                persist.tile([P, CB], bf16, name="esum1", tag="esum1"),
            ]
            zb = persist.tile([P, 1], f32, tag="zb")
            b_ln4 = persist.tile([P, 1], f32, tag="b_ln4")
            mask2 = persist.tile([P, 2], bf16, tag="mask2")
            dxin = persist.tile([16, D], bf16, tag="dxin")
            dxout = persist.tile([P, 16], bf16, tag="dxout")

            rva = dg[:, 0:RT]
            rvb = dg[:, RT:2 * RT]
            invi = dg[:, 2 * RT:3 * RT]
            n2c_out = dg[:, 3 * RT:5 * RT]       # [n2a | n2b]

            # ---- loads: all on the sync queue so arrival order is
            # deterministic (a second-queue load racing the cj halves to
            # the DMA engines stalls the in-order ACT square chain) ----
            nc.sync.dma_start(out=cjb, in_=cb_t)
            nc.sync.dma_start(out=cja, in_=ca_t)
            nc.scalar.dma_start(out=aib, in_=ai_t)

            # ---- tiny constants ----
            nc.vector.memset(zb, 0.0)
            nc.vector.memset(b_ln4, LN4)
            nc.vector.memset(mask2, 1.0)
            nc.vector.memset(dxin, 0.0)

            dummy_inst = nc.sync.dma_start_transpose(out=dxout, in_=dxin)

            # ---- squares on ACT (a DVE TT squaring one operand runs at
            # half rate; ACT is idle in the preamble) ----
            nc.scalar.activation(sqb, cjb, AF.Square)
            nc.scalar.activation(sqa, cja, AF.Square)

            # ---- DVE reds + scales (order = queue order), b first ----
            nc.vector.tensor_reduce(out=n2b, in_=sqb, axis=AX.X, op=ALU.add)
            nc.vector.tensor_reduce(out=n2a, in_=sqa, axis=AX.X, op=ALU.add)

            lgb = scratch.tile([P, RT], f32, tag="lgb")
            nc.scalar.activation(lgb, n2b, AF.Ln, bias=zb)
            nc.scalar.activation(inv4b[:, :, 0], lgb, AF.Exp,
                                 scale=-0.5, bias=b_ln4)
            nc.scalar.activation(sqi, aib, AF.Square)
            lga = scratch.tile([P, RT], f32, tag="lga")
            nc.scalar.activation(lga, n2a, AF.Ln, bias=zb)
            nc.scalar.activation(inv4a[:, :, 0], lga, AF.Exp,
                                 scale=-0.5, bias=b_ln4)

            # scale_b on the idle GPSIMD (slower per-op, but parallel to
            # DVE, so scale_a starts ~1us earlier and both transposes
            # shift left)
            a_ap, b_ap = broadcast_tensor_aps(cjb, inv4b)
            sc_b = nc.gpsimd.tensor_tensor(out=zcb, in0=a_ap, in1=b_ap,
                                           op=ALU.mult)
            a_ap, b_ap = broadcast_tensor_aps(cja, inv4a)
            sc_a = nc.vector.tensor_tensor(out=zca, in0=a_ap, in1=b_ap,
                                           op=ALU.mult)

            # invi gates only the EXPs (~2us later than the transposes):
            # dep-force red_i after the scales so the scheduler can't hoist
            # it into the scale->transpose critical chain
            red_i = nc.vector.tensor_reduce(out=n2i, in_=sqi, axis=AX.X,
                                            op=ALU.add)
            add_dep_helper(red_i.ins, sc_a.ins, False, "red_i late")

            lgi = scratch.tile([P, RT], f32, tag="lgi")
            nc.scalar.activation(lgi, n2i, AF.Ln, bias=zb)
            nc.scalar.activation(invi, lgi, AF.Exp, scale=-0.5, bias=zb)

            # n2c ships to host (diagonal needs |z_j| of own rows);
            # dep-forced late so they can't delay the scale chain on DVE
            cp1 = nc.vector.tensor_copy(n2c_out[:, 0:RT], n2a)
            add_dep_helper(cp1.ins, sc_a.ins, False, "n2 copy late")
            cp2 = nc.vector.tensor_copy(n2c_out[:, RT:2 * RT], n2b)
            add_dep_helper(cp2.ins, sc_a.ins, False, "n2 copy late")

            # ---- transposes (xbar serial, ~1.25us FLAT per call up to
            # 8-tile size -> fewer, bigger calls win): aibT asap, then
            # the two col halves ----
            t1 = nc.sync.dma_start_transpose(out=aibT, in_=aib)
            add_dep_helper(t1.ins, dummy_inst.ins, False, "xpose after dummy")
            t2 = nc.sync.dma_start_transpose(out=zcbT, in_=zcb)
            add_dep_helper(t2.ins, dummy_inst.ins, False, "xpose after dummy")
            t3 = nc.sync.dma_start_transpose(out=zcaT, in_=zca)
            add_dep_helper(t3.ins, dummy_inst.ins, False, "xpose after dummy")

            # ---- diag elementwise on GPSIMD; dep-forced after the scales
            # (a GP mult reading cjb concurrently with scale_b's read was
            # costing ~0.7us of SBUF port contention) ----
            dda = scratch.tile([P, RT, D], bf16, tag="dda")
            ddb = scratch.tile([P, RT, D], bf16, tag="ddb")
            nc.gpsimd.tensor_mul(dda, aib, cja)
            nc.gpsimd.tensor_mul(ddb, aib, cjb)

            zcaT_f = zcaT.rearrange("p t d -> p (t d)")
            zcbT_f = zcbT.rearrange("p t d -> p (t d)")
            movs = [zcaT_f[:, 0:512], zcaT_f[:, 512:1024],
                    zcbT_f[:, 0:512], zcbT_f[:, 512:1024]]

            # ---- main loop: 8 chunks (one row tile each) ----
            eb_last = None
            for k in range(RT):
                ps = psmain.tile([P, CB], f32, tag="ps")
                for q in (2, 3, 0, 1):      # b-cols first (transposed first)
                    nc.tensor.matmul(
                        ps[:, q * 512:(q + 1) * 512],
                        aibT[:, k, :],
                        movs[q],
                        start=True,
                        stop=True,
                    )
                eb = ebuf.tile([P, CB], bf16, tag="eb")
                eb_last = eb
                # chunk 7 skips accum_out: its aux accumulator-read would
                # delay eb_7's ready-sem (the tail's critical input); its
                # rowsum comes from a DVE reduce in the tail (DVE is idle)
                nc.scalar.activation(
                    eb, ps, AF.Exp,
                    scale=invi[:, k:k + 1],
                    bias=zb,
                    accum_out=(rs_sb[:, k:k + 1] if k < RT - 1 else None),
                )
                if k == 0:
                    nc.vector.tensor_copy(esum[0], eb)        # 4x
                elif k < RT - 1:
                    nc.vector.tensor_tensor(
                        out=esum[k % 2], in0=eb, in1=esum[(k + 1) % 2],
                        op=ALU.add,
                    )                                          # 2x
                if k == 2:
                    # diag reduces into DVE loop slack; dep-forced after
                    # red_i so the scheduler can't hoist them earlier
                    r1 = nc.vector.tensor_reduce(out=rva, in_=dda,
                                                 axis=AX.X, op=ALU.add)
                    add_dep_helper(r1.ins, red_i.ins, False, "diag late")
                if k == 4:
                    r2 = nc.vector.tensor_reduce(out=rvb, in_=ddb,
                                                 axis=AX.X, op=ALU.add)
                    add_dep_helper(r2.ins, red_i.ins, False, "diag late")

            es_fin = esum[(RT - 2) % 2]     # chain through chunk 6

            # chunk 7's rowsum on DVE, parallel to the mask matmuls
            nc.vector.tensor_reduce(out=rs_sb[:, RT - 1:RT], in_=eb_last,
                                    axis=AX.X, op=ALU.add)

            # ---- tail: colsum partials via accumulating mask matmuls;
            # q-th block on PSUM partitions 32q, free 0:512 ----
            psR_full = psmain.tile([P, CB], f32, tag="ps")
            for q in range(4):
                nc.tensor.matmul(
                    psR_full[32 * q:32 * q + 2, 0:512],
                    mask2,
                    es_fin[:, q * 512:(q + 1) * 512],
                    start=True,
                    stop=False,
                    tile_position=(0, 32 * q),
                )
            for q in range(4):
                nc.tensor.matmul(
                    psR_full[32 * q:32 * q + 2, 0:512],
                    mask2,
                    eb_last[:, q * 512:(q + 1) * 512],
                    start=False,
                    stop=True,
                    tile_position=(0, 32 * q),
                )
            nc.scalar.activation(cs_sb, psR_full[:, 0:512], AF.Copy)  # ->bf16

            # cs is the end-critical output: its issue parks on the (empty)
            # sync queue waiting the copy; dg/rs issue from the ACT queue
            # after the copy, when their data is long ready.
            nc.sync.dma_start(out=out_cs[:, :], in_=cs_sb)
            nc.scalar.dma_start(out=out_dg[:, :], in_=dg)
            nc.scalar.dma_start(out=out_rs[:, :], in_=rs_sb)

    return nc


def _split_multi_waits(bir: bytes) -> bytes:
    """The walrus build in this container accepts only ONE sync-wait per
    compute/DMA instruction. Tile emits up to three. Move all but one wait
    onto standalone EventSemaphore instructions inserted just before the
    offender on the same engine queue."""
    import json

    d = json.loads(bir)
    n_split = 0
    for fn in d["functions"]:
        for blk in fn["blocks"]:
            new_insts = []
            for ins in blk["instructions"]:
                si = ins.get("sync_info")
                waits = (si or {}).get("on_wait") or []
                if len(waits) > 1:
                    for w in waits[:-1]:
                        ev = {
                            "debug": ins.get("debug", 0),
                            "engine": ins["engine"],
                            "ins": [],
                            "outs": [],
                            "name": f"{ins['name']}_wsplit{n_split}",
                            "opcode": "EventSemaphore",
                            "sync_info": {"on_update": [], "on_wait": [w]},
                        }
                        n_split += 1
                        new_insts.append(ev)
                    si["on_wait"] = [waits[-1]]
                new_insts.append(ins)
            blk["instructions"] = new_insts
    return json.dumps(d).encode()


def kernel(emb_i: np.ndarray, emb_j: np.ndarray) -> np.ndarray:
    import ml_dtypes
    from concourse.bass_utils import run_bass_kernel_spmd

    if "nc" not in _cache:
        nc = _build_bass()
        fixed = _split_multi_waits(nc.to_json_bytes())
        nc.to_json_bytes = lambda: fixed
        _cache["nc"] = nc
    nc = _cache["nc"]

    bf = ml_dtypes.bfloat16
    emb_i = np.ascontiguousarray(emb_i, dtype=np.float32).astype(bf)
    emb_j = np.ascontiguousarray(emb_j, dtype=np.float32).astype(bf)
    in_maps = []
    for c in range(NCORES):
        rb, ch = c // 2, c % 2
        in_maps.append(
            {
                "emb_i_blk": emb_i[rb * RB:(rb + 1) * RB],
                "emb_j_ca": emb_j[ch * CB:ch * CB + RB],
                "emb_j_cb": emb_j[ch * CB + RB:(ch + 1) * CB],
            }
        )

    import os

    trace = bool(os.environ.get("KERNEL_TRACE"))
    res = run_bass_kernel_spmd(
        nc, in_maps, core_ids=list(range(NCORES)), trace=trace
    )
    _cache["last_res"] = res

    # ---- host combine ----
    rs_total = np.zeros(B, dtype=np.float64)
    cs_total = np.zeros(B, dtype=np.float64)
    dtot = np.float64(0.0)
    for c, r in enumerate(res.results):
        rb, ch = c // 2, c % 2
        # rowsum [128, 8]: (p, k) -> local row p*8+k
        rs_total[rb * RB:(rb + 1) * RB] += (
            r["rowsum"].astype(np.float64).reshape(RB)
        )
        # colsum [128, 512]: q-th 512-block on partitions 32q..32q+1;
        # block free j = t_l*128 + p -> local col
        #   1024*(q//2) + p*8 + 4*(q%2) + t_l
        co = r["colsum"].astype(np.float64)
        for q in range(4):
            blk = 0.5 * (co[32 * q] + co[32 * q + 1])        # [512]
            half, sub = q // 2, q % 2
            dst = cs_total[ch * CB + half * RB:ch * CB + half * RB + RB]
            dst.reshape(P, 2, 4)[:, sub, :] += blk.reshape(4, P).T
        if rb // 2 == ch:
            # this core's emb_i row block lies inside its cj col block
            d = r["diag"].astype(np.float64)
            iv = d[:, 2 * RT:3 * RT].reshape(RB)
            half = rb % 2
            rv = d[:, half * RT:(half + 1) * RT].reshape(RB)
            n2o = d[:, (3 + half) * RT:(4 + half) * RT].reshape(RB)
            # pos/T = 4 * rvec * invi / sqrt(n2o); contributes -2*pos/T
            dtot += np.sum(-8.0 * rv * iv / np.sqrt(n2o))
    total = dtot + np.log(rs_total).sum() + np.log(cs_total).sum()
    loss = total / (2 * B)
    return np.array(loss, dtype=np.float32)


# revision 49
# speedup vs baseline: 1.0389x; 1.0389x over previous
"""Trainium2 Bass kernel for NT-Xent style contrastive loss (v5).

Math (B=4096, D=128, T=0.25), z = row-normalized emb:
  S = z_i @ z_j^T   [B, B]
  loss = (1/2B) * sum_r [ -2*S[r,r]/T + ln(sum_c exp(S[r,c]/T))
                                      + ln(sum_c exp(S[c,r]/T)) ]

Sharding: core (rb, ch), rb = core//2, ch = core%2: rows rb*1024 of emb_i,
cols ch*2048 of emb_j.  Inputs are cast to bf16 ON THE HOST (input
quantization, ~0.4% per element, far inside the 2e-2 tolerance): halves
the DMA wire time, feeds the matmuls directly, and makes every stats op
2x-mode eligible.  All tensors use the "(p t) d" 8-row tiling (partition
g//8, tile g%8, 2KB contiguous descriptors); outputs are unpermuted on
the host.  emb_j's block is loaded as two 1024-row halves with the same
tiling as emb_i's block, so the diagonal dot products are computed
against cj directly (each row block equals one cj half on cores
0/2/5/7) and the column norms n2c double as the diagonal |z_j| norms.

Orientation: ps chunk k = [128 r (row-tile k), 2048 c free], 8 chunks.
  stationary = aibT tile k (RAW bf16 emb_i, transposed straight off the
               load; row stats are off the matmul critical path)
  moving     = zcjT halves (cols pre-scaled by 4/|c|, absorbing 1/T)
The EXP applies the row norm via its per-partition scale AP
(scale = invi[:,k]) and its accumulator emits the ROW SUMS directly
(accum_out -> rs[:, k]).  Column-sum partials: bf16 TT esum chain
(DVE 2x, chunks 1-6) + accumulating mask matmuls over {esum, eb_7},
stacked on PSUM partitions 32q so one free-512 ACT copy extracts them.

Engine split: DVE sq_a/red_a/sq_i/red_b/scale_a/scale_b/red_i then the
esum chain (diag reduces dep-forced into loop slack); GP sq_b + diag
mults; ACT only ln/exp stat scalars + the 8 big EXPs + cs copy.
"""

import numpy as np

B = 4096
D = 128
P = 128
NCORES = 8
RB = 1024                  # rows per core
CB = 2048                  # cols per core
RT = RB // P               # 8 row tiles
TEMP = 0.25
LN4 = float(np.log(4.0))

_cache = {}


def _build_bass():
    import concourse.bass as bass
    import concourse.mybir as mybir
    import concourse.tile as tile
    from concourse.bass import broadcast_tensor_aps
    from concourse.tile_rust import add_dep_helper

    f32 = mybir.dt.float32
    bf16 = mybir.dt.bfloat16
    AF = mybir.ActivationFunctionType
    ALU = mybir.AluOpType
    AX = mybir.AxisListType

    nc = bass.Bass("TRN2")
    ai_d = nc.dram_tensor("emb_i_blk", [RB, D], bf16, kind="ExternalInput")
    ca_d = nc.dram_tensor("emb_j_ca", [RB, D], bf16, kind="ExternalInput")
    cb_d = nc.dram_tensor("emb_j_cb", [RB, D], bf16, kind="ExternalInput")
    out_rs = nc.dram_tensor("rowsum", [P, RT], f32, kind="ExternalOutput")
    out_cs = nc.dram_tensor("colsum", [P, 512], bf16, kind="ExternalOutput")
    out_dg = nc.dram_tensor("diag", [P, 5 * RT], f32, kind="ExternalOutput")

    ai_t = ai_d.rearrange("(p t) d -> p t d", p=P)   # row g = p*8 + t
    ca_t = ca_d.rearrange("(p t) d -> p t d", p=P)   # col g = p*8 + t
    cb_t = cb_d.rearrange("(p t) d -> p t d", p=P)   # col g = 1024 + p*8 + t

    with tile.TileContext(nc) as tc:
        with (
            tc.tile_pool(name="persist", bufs=1) as persist,
            tc.tile_pool(name="scratch", bufs=4) as scratch,
            tc.tile_pool(name="ebuf", bufs=2) as ebuf,
            tc.tile_pool(name="psmain", bufs=2, space="PSUM") as psmain,
        ):
            aib = persist.tile([P, RT, D], bf16, tag="aib")
            cja = persist.tile([P, RT, D], bf16, tag="cja")
            cjb = persist.tile([P, RT, D], bf16, tag="cjb")
            aibT = persist.tile([P, RT, D], bf16, tag="aibT")
            sqi = persist.tile([P, RT, D], bf16, tag="sqi")
            sqa = persist.tile([P, RT, D], bf16, tag="sqa")
            sqb = persist.tile([P, RT, D], bf16, tag="sqb")
            zca = persist.tile([P, RT, D], bf16, tag="zca")
            zcb = persist.tile([P, RT, D], bf16, tag="zcb")
            zcaT = persist.tile([P, RT, D], bf16, tag="zcaT")
            zcbT = persist.tile([P, RT, D], bf16, tag="zcbT")
            n2a = persist.tile([P, RT], f32, tag="n2a")
            n2b = persist.tile([P, RT], f32, tag="n2b")
            n2i = persist.tile([P, RT], f32, tag="n2i")
            inv4a = persist.tile([P, RT, 1], bf16, tag="inv4a")
            inv4b = persist.tile([P, RT, 1], bf16, tag="inv4b")
            dg = persist.tile([P, 5 * RT], f32, tag="dg")
            rs_sb = persist.tile([P, RT], f32, tag="rs_sb")
            cs_sb = persist.tile([P, 512], bf16, tag="cs_sb")
            esum = [
                persist.tile([P, CB], bf16, name="esum0", tag="esum0"),
                persist.tile([P, CB], bf16, name="esum1", tag="esum1"),
            ]
            zb = persist.tile([P, 1], f32, tag="zb")
            b_ln4 = persist.tile([P, 1], f32, tag="b_ln4")
            mask2 = persist.tile([P, 2], bf16, tag="mask2")
            dxin = persist.tile([16, D], bf16, tag="dxin")
            dxout = persist.tile([P, 16], bf16, tag="dxout")

            rva = dg[:, 0:RT]
            rvb = dg[:, RT:2 * RT]
            invi = dg[:, 2 * RT:3 * RT]
            n2c_out = dg[:, 3 * RT:5 * RT]       # [n2a | n2b]

            # ---- loads: all on the sync queue so arrival order is
            # deterministic (a second-queue load racing the cj halves to
            # the DMA engines stalls the in-order ACT square chain) ----
            nc.sync.dma_start(out=cjb, in_=cb_t)
            nc.sync.dma_start(out=cja, in_=ca_t)
            nc.scalar.dma_start(out=aib, in_=ai_t)

            # ---- tiny constants ----
            nc.vector.memset(zb, 0.0)
            nc.vector.memset(b_ln4, LN4)
            nc.vector.memset(mask2, 1.0)
            nc.vector.memset(dxin, 0.0)

            dummy_inst = nc.sync.dma_start_transpose(out=dxout, in_=dxin)

            # ---- squares on ACT (a DVE TT squaring one operand runs at
            # half rate; ACT is idle in the preamble) ----
            nc.scalar.activation(sqb, cjb, AF.Square)
            nc.scalar.activation(sqa, cja, AF.Square)

            # ---- DVE reds + scales (order = queue order), b first ----
            nc.vector.tensor_reduce(out=n2b, in_=sqb, axis=AX.X, op=ALU.add)
            nc.vector.tensor_reduce(out=n2a, in_=sqa, axis=AX.X, op=ALU.add)

            lgb = scratch.tile([P, RT], f32, tag="lgb")
            nc.scalar.activation(lgb, n2b, AF.Ln, bias=zb)
            nc.scalar.activation(inv4b[:, :, 0], lgb, AF.Exp,
                                 scale=-0.5, bias=b_ln4)
            nc.scalar.activation(sqi, aib, AF.Square)
            lga = scratch.tile([P, RT], f32, tag="lga")
            nc.scalar.activation(lga, n2a, AF.Ln, bias=zb)
            nc.scalar.activation(inv4a[:, :, 0], lga, AF.Exp,
                                 scale=-0.5, bias=b_ln4)

            a_ap, b_ap = broadcast_tensor_aps(cjb, inv4b)
            sc_b = nc.vector.tensor_tensor(out=zcb, in0=a_ap, in1=b_ap,
                                           op=ALU.mult)
            a_ap, b_ap = broadcast_tensor_aps(cja, inv4a)
            sc_a = nc.vector.tensor_tensor(out=zca, in0=a_ap, in1=b_ap,
                                           op=ALU.mult)

            # invi gates only the EXPs (~2us later than the transposes):
            # dep-force red_i after the scales so the scheduler can't hoist
            # it into the scale->transpose critical chain
            red_i = nc.vector.tensor_reduce(out=n2i, in_=sqi, axis=AX.X,
                                            op=ALU.add)
            add_dep_helper(red_i.ins, sc_a.ins, False, "red_i late")

            lgi = scratch.tile([P, RT], f32, tag="lgi")
            nc.scalar.activation(lgi, n2i, AF.Ln, bias=zb)
            nc.scalar.activation(invi, lgi, AF.Exp, scale=-0.5, bias=zb)

            # n2c ships to host (diagonal needs |z_j| of own rows);
            # dep-forced late so they can't delay the scale chain on DVE
            cp1 = nc.vector.tensor_copy(n2c_out[:, 0:RT], n2a)
            add_dep_helper(cp1.ins, sc_a.ins, False, "n2 copy late")
            cp2 = nc.vector.tensor_copy(n2c_out[:, RT:2 * RT], n2b)
            add_dep_helper(cp2.ins, sc_a.ins, False, "n2 copy late")

            # ---- transposes (xbar serial, ~1.25us FLAT per call up to
            # 8-tile size -> fewer, bigger calls win): aibT asap, then
            # the two col halves ----
            t1 = nc.sync.dma_start_transpose(out=aibT, in_=aib)
            add_dep_helper(t1.ins, dummy_inst.ins, False, "xpose after dummy")
            t2 = nc.sync.dma_start_transpose(out=zcbT, in_=zcb)
            add_dep_helper(t2.ins, dummy_inst.ins, False, "xpose after dummy")
            t3 = nc.sync.dma_start_transpose(out=zcaT, in_=zca)
            add_dep_helper(t3.ins, dummy_inst.ins, False, "xpose after dummy")

            # ---- diag elementwise on GPSIMD; dep-forced after the scales
            # (a GP mult reading cjb concurrently with scale_b's read was
            # costing ~0.7us of SBUF port contention) ----
            dda = scratch.tile([P, RT, D], bf16, tag="dda")
            ddb = scratch.tile([P, RT, D], bf16, tag="ddb")
            nc.gpsimd.tensor_mul(dda, aib, cja)
            nc.gpsimd.tensor_mul(ddb, aib, cjb)

            zcaT_f = zcaT.rearrange("p t d -> p (t d)")
            zcbT_f = zcbT.rearrange("p t d -> p (t d)")
            movs = [zcaT_f[:, 0:512], zcaT_f[:, 512:1024],
                    zcbT_f[:, 0:512], zcbT_f[:, 512:1024]]

            # ---- main loop: 8 chunks (one row tile each) ----
            eb_last = None
            for k in range(RT):
                ps = psmain.tile([P, CB], f32, tag="ps")
                for q in (2, 3, 0, 1):      # b-cols first (transposed first)
                    nc.tensor.matmul(
                        ps[:, q * 512:(q + 1) * 512],
                        aibT[:, k, :],
                        movs[q],
                        start=True,
                        stop=True,
                    )
                eb = ebuf.tile([P, CB], bf16, tag="eb")
                eb_last = eb
                # chunk 7 skips accum_out: its aux accumulator-read would
                # delay eb_7's ready-sem (the tail's critical input); its
                # rowsum comes from a DVE reduce in the tail (DVE is idle)
                nc.scalar.activation(
                    eb, ps, AF.Exp,
                    scale=invi[:, k:k + 1],
                    bias=zb,
                    accum_out=(rs_sb[:, k:k + 1] if k < RT - 1 else None),
                )
                if k == 0:
                    nc.vector.tensor_copy(esum[0], eb)        # 4x
                elif k < RT - 1:
                    nc.vector.tensor_tensor(
                        out=esum[k % 2], in0=eb, in1=esum[(k + 1) % 2],
                        op=ALU.add,
                    )                                          # 2x
                if k == 2:
                    # diag reduces into DVE loop slack; dep-forced after
                    # red_i so the scheduler can't hoist them earlier
                    r1 = nc.vector.tensor_reduce(out=rva, in_=dda,
                                                 axis=AX.X, op=ALU.add)
                    add_dep_helper(r1.ins, red_i.ins, False, "diag late")
                if k == 4:
                    r2 = nc.vector.tensor_reduce(out=rvb, in_=ddb,
                                                 axis=AX.X, op=ALU.add)
                    add_dep_helper(r2.ins, red_i.ins, False, "diag late")

            es_fin = esum[(RT - 2) % 2]     # chain through chunk 6

            # chunk 7's rowsum on DVE, parallel to the mask matmuls
            nc.vector.tensor_reduce(out=rs_sb[:, RT - 1:RT], in_=eb_last,
                                    axis=AX.X, op=ALU.add)

            # ---- tail: colsum partials via accumulating mask matmuls;
            # q-th block on PSUM partitions 32q, free 0:512 ----
            psR_full = psmain.tile([P, CB], f32, tag="ps")
            for q in range(4):
                nc.tensor.matmul(
                    psR_full[32 * q:32 * q + 2, 0:512],
                    mask2,
                    es_fin[:, q * 512:(q + 1) * 512],
                    start=True,
                    stop=False,
                    tile_position=(0, 32 * q),
                )
            for q in range(4):
                nc.tensor.matmul(
                    psR_full[32 * q:32 * q + 2, 0:512],
                    mask2,
                    eb_last[:, q * 512:(q + 1) * 512],
                    start=False,
                    stop=True,
                    tile_position=(0, 32 * q),
                )
            nc.scalar.activation(cs_sb, psR_full[:, 0:512], AF.Copy)  # ->bf16

            # cs is the end-critical output: its issue parks on the (empty)
            # sync queue waiting the copy; dg/rs issue from the ACT queue
            # after the copy, when their data is long ready.
            nc.sync.dma_start(out=out_cs[:, :], in_=cs_sb)
            nc.scalar.dma_start(out=out_dg[:, :], in_=dg)
            nc.scalar.dma_start(out=out_rs[:, :], in_=rs_sb)

    return nc


def _split_multi_waits(bir: bytes) -> bytes:
    """The walrus build in this container accepts only ONE sync-wait per
    compute/DMA instruction. Tile emits up to three. Move all but one wait
    onto standalone EventSemaphore instructions inserted just before the
    offender on the same engine queue."""
    import json

    d = json.loads(bir)
    n_split = 0
    for fn in d["functions"]:
        for blk in fn["blocks"]:
            new_insts = []
            for ins in blk["instructions"]:
                si = ins.get("sync_info")
                waits = (si or {}).get("on_wait") or []
                if len(waits) > 1:
                    for w in waits[:-1]:
                        ev = {
                            "debug": ins.get("debug", 0),
                            "engine": ins["engine"],
                            "ins": [],
                            "outs": [],
                            "name": f"{ins['name']}_wsplit{n_split}",
                            "opcode": "EventSemaphore",
                            "sync_info": {"on_update": [], "on_wait": [w]},
                        }
                        n_split += 1
                        new_insts.append(ev)
                    si["on_wait"] = [waits[-1]]
                new_insts.append(ins)
            blk["instructions"] = new_insts
    return json.dumps(d).encode()


def kernel(emb_i: np.ndarray, emb_j: np.ndarray) -> np.ndarray:
    import ml_dtypes
    from concourse.bass_utils import run_bass_kernel_spmd

    if "nc" not in _cache:
        nc = _build_bass()
        fixed = _split_multi_waits(nc.to_json_bytes())
        nc.to_json_bytes = lambda: fixed
        _cache["nc"] = nc
    nc = _cache["nc"]

    bf = ml_dtypes.bfloat16
    emb_i = np.ascontiguousarray(emb_i, dtype=np.float32).astype(bf)
    emb_j = np.ascontiguousarray(emb_j, dtype=np.float32).astype(bf)
    in_maps = []
    for c in range(NCORES):
        rb, ch = c // 2, c % 2
        in_maps.append(
            {
                "emb_i_blk": emb_i[rb * RB:(rb + 1) * RB],
                "emb_j_ca": emb_j[ch * CB:ch * CB + RB],
                "emb_j_cb": emb_j[ch * CB + RB:(ch + 1) * CB],
            }
        )

    import os

    trace = bool(os.environ.get("KERNEL_TRACE"))
    res = run_bass_kernel_spmd(
        nc, in_maps, core_ids=list(range(NCORES)), trace=trace
    )
    _cache["last_res"] = res

    # ---- host combine ----
    rs_total = np.zeros(B, dtype=np.float64)
    cs_total = np.zeros(B, dtype=np.float64)
    dtot = np.float64(0.0)
    for c, r in enumerate(res.results):
        rb, ch = c // 2, c % 2
        # rowsum [128, 8]: (p, k) -> local row p*8+k
        rs_total[rb * RB:(rb + 1) * RB] += (
            r["rowsum"].astype(np.float64).reshape(RB)
        )
        # colsum [128, 512]: q-th 512-block on partitions 32q..32q+1;
        # block free j = t_l*128 + p -> local col
        #   1024*(q//2) + p*8 + 4*(q%2) + t_l
        co = r["colsum"].astype(np.float64)
        for q in range(4):
            blk = 0.5 * (co[32 * q] + co[32 * q + 1])        # [512]
            half, sub = q // 2, q % 2
            dst = cs_total[ch * CB + half * RB:ch * CB + half * RB + RB]
            dst.reshape(P, 2, 4)[:, sub, :] += blk.reshape(4, P).T
        if rb // 2 == ch:
            # this core's emb_i row block lies inside its cj col block
            d = r["diag"].astype(np.float64)
            iv = d[:, 2 * RT:3 * RT].reshape(RB)
            half = rb % 2
            rv = d[:, half * RT:(half + 1) * RT].reshape(RB)
            n2o = d[:, (3 + half) * RT:(4 + half) * RT].reshape(RB)
            # pos/T = 4 * rvec * invi / sqrt(n2o); contributes -2*pos/T
            dtot += np.sum(-8.0 * rv * iv / np.sqrt(n2o))
    total = dtot + np.log(rs_total).sum() + np.log(cs_total).sum()
    loss = total / (2 * B)
    return np.array(loss, dtype=np.float32)
